# revision 10
# baseline (speedup 1.0000x reference)
"""Trainium2 Bass kernel for nn_AttnAligner.

Strategy: data-parallel over the batch (8 samples -> 8 NeuronCores), one
sample per core.  The (bs, ps, ll) scatter matrix is never materialized:

    alg.T @ emb_w  ==  attn[b].T @ emb_w[tgt[b]]        (segment_sum is linear)

so the host gathers the 64 embedding rows per sample (zeroing rows where
tgt == 0, which reproduces the `alg[:, :, 0] = 0` vocab-0 fill), and the
device runs: tiny matmul -> 4-layer transformer encoder -> (256, 16000)
output projection.  proj_w is transposed on the host so the device only
does k-major streaming matmuls.  Matmuls use float32r (full-rate fp32)
with K padded to 128; attention uses the unnormalized-exp trick (softmax
denominator folded into the output-copy scale) and computes both s and
s^T on the PE so no per-head transposes are needed.
"""

import numpy as np

BS, LS, LL = 8, 64, 256
PS, D, NL, NH = 16000, 512, 4, 8
DH = D // NH          # 64
DF = 4 * D            # 2048
P = 128
N_IB = LL // P        # 2   seq-partition blocks
N_KB = D // P         # 4   d-partition blocks
N_FB = DF // P        # 16  ffn-partition blocks
PBW = 500             # proj column chunk (<=512 psum free dim)
N_PB = PS // PBW      # 32
F32 = np.float32


def _build_bass(use_mask_bias: bool, use_ln_gb: bool, use_b2: bool):
    import concourse.bass as bass
    import concourse.mybir as mybir
    import concourse.tile as tile
    from concourse.masks import make_identity
    from contextlib import ExitStack
    from bass_rust import ScopedClock

    f32 = mybir.dt.float32
    f32r = mybir.dt.float32r
    AX = mybir.AxisListType
    OP = mybir.AluOpType
    ACT = mybir.ActivationFunctionType

    class PatchedTC(tile.TileContext):
        """The walrus build in this container rejects >2 sync waits on the
        kernel-tail Drain.  Emit the waits as individual EVSEM wait
        instructions instead, then a waitless drain."""

        def _drain_and_barrier(self, tick_clock, wait_clock):
            dummy = mybir.InstDrain(
                name=f"I-{self.nc.next_id()}", engine=mybir.EngineType.SP
            )
            wait_clock.add_sem_waits(
                dummy, ScopedClock({None: tick_clock.global_clock})
            )
            waits = dummy.sync_info.on_wait if dummy.sync_info is not None else []
            assert self.sems is not None
            handles = {h.name: h for h in self.sems.allocated().values()}
            for w in waits:
                self.nc.sync.wait_ge(handles[w.ant_name], w.wait_value)
            self.nc.sync.drain()
            self.nc.all_engine_barrier()
            popped = self.nc._tile_sem_poison_stack.pop()
            assert popped is self._sem_poison
            self.nc.clear_and_free_semaphores(list(self.sems.allocated().values()))
            self.nc.all_engine_barrier()

    nc = bass.Bass("TRN2", target_bir_lowering=False, debug=False)

    def din(name, shape, dt=None):
        return nc.dram_tensor(name, list(shape), dt or f32,
                              kind="ExternalInput").ap()

    attn_d = din("attn", (LS, LL), f32r)
    embg_d = din("embg", (LS, D), f32r)
    pe_d = din("pe", (N_IB, P, D))
    wq_d = din("wq", (NL, P, N_KB, D), f32r)
    wk_d = din("wk", (NL, P, N_KB, D), f32r)
    wv_d = din("wv", (NL, P, N_KB, D), f32r)
    wo_d = din("wo", (NL, P, N_KB, D), f32r)
    w1_d = din("w1", (NL, N_FB, P, N_KB, P), f32r)
    w2_d = din("w2", (NL, N_FB, P, D), f32r)
    b1_d = din("b1", (NL, P, N_FB))
    if use_ln_gb:
        lnw_d = din("lnw", (NL, 4, P, D))   # ln1_g, ln1_b, ln2_g, ln2_b replicated
    if use_b2:
        b2r_d = din("b2r", (NL, P, D))
    projr_d = din("projr", (N_PB, P, N_KB, PBW), f32r)
    if use_mask_bias:
        maskb_d = din("maskb", (P, LL))      # (1-mask)*-1e9 replicated over rows
        maskbT_d = din("maskbT", (N_IB, P, 1))
    out_d = nc.dram_tensor("out", [LL, PS], f32, kind="ExternalOutput").ap()

    with PatchedTC(nc) as tc, ExitStack() as stk:
        const = stk.enter_context(tc.tile_pool(name="const", bufs=1))
        sb = stk.enter_context(tc.tile_pool(name="sb", bufs=2))
        pp_big = stk.enter_context(tc.tile_pool(name="pp_big", bufs=2, space="PSUM"))
        pp_med = stk.enter_context(tc.tile_pool(name="pp_med", bufs=2, space="PSUM"))
        pp_o = stk.enter_context(tc.tile_pool(name="pp_o", bufs=2, space="PSUM"))
        pp_tr = stk.enter_context(tc.tile_pool(name="pp_tr", bufs=2, space="PSUM"))

        ident = const.tile([P, P], f32, tag="ident", name="ident")
        make_identity(nc, ident)
        epsb = const.tile([P, 1], f32, tag="epsb", name="epsb")
        nc.vector.memset(epsb[:], 1e-5)

        # persistent zero-padded per-head stationaries: rows of the "other"
        # head stay zero forever, only the live half is rewritten per layer
        qTp = [const.tile([P, LL], f32r, tag=f"qTp{h}", name=f"qTp{h}")
               for h in range(NH)]
        kTp = [const.tile([P, LL], f32r, tag=f"kTp{h}", name=f"kTp{h}")
               for h in range(NH)]
        for h in range(NH):
            nc.vector.memset(qTp[h][:].bitcast(f32), 0.0)
            nc.vector.memset(kTp[h][:].bitcast(f32), 0.0)

        pe_sb = [const.tile([P, D], f32, tag=f"pe{ib}", name=f"pe{ib}")
                 for ib in range(N_IB)]
        for ib in range(N_IB):
            nc.sync.dma_start(pe_sb[ib][:], pe_d[ib])
        attn_sb = const.tile([LS, LL], f32r, tag="attn", name="attn")
        nc.sync.dma_start(attn_sb[:], attn_d[:])
        embg_sb = const.tile([LS, D], f32r, tag="embg", name="embg")
        nc.sync.dma_start(embg_sb[:], embg_d[:])
        if use_mask_bias:
            maskb_sb = const.tile([P, LL], f32, tag="maskb", name="maskb")
            nc.sync.dma_start(maskb_sb[:], maskb_d[:])
            maskbT_sb = [const.tile([P, 1], f32, tag=f"maskbT{j}", name=f"maskbT{j}")
                         for j in range(N_IB)]
            for j in range(N_IB):
                nc.sync.dma_start(maskbT_sb[j][:], maskbT_d[j])

        def transpose_into(src_tiles, tag):
            """src: list of 2 [128, 512] seq-major f32 tiles -> 4 [128, 256]
            d-major f32r tiles."""
            dst = [sb.tile([P, LL], f32r, tag=f"{tag}{kb}", name=f"{tag}{kb}",
                           bufs=2) for kb in range(N_KB)]
            for kb in range(N_KB):
                for ib in range(N_IB):
                    pt = pp_tr.tile([P, P], f32, tag="tr", name="tr")
                    nc.tensor.transpose(pt[:], src_tiles[ib][:, kb * P:(kb + 1) * P],
                                        ident[:])
                    nc.vector.tensor_copy(dst[kb][:, ib * P:(ib + 1) * P], pt[:])
            return dst

        def layer_norm(ps_in, resid, gw, bw, out_tag, bufs):
            """y = LN(psum + resid); var via E[x^2] - mu^2."""
            xr = sb.tile([P, D], f32, tag=f"{out_tag}_pre", name=f"{out_tag}_pre",
                         bufs=2)
            nc.vector.tensor_tensor(xr[:], ps_in[:], resid[:], OP.add)
            sq = sb.tile([P, D], f32, tag="sq", name="sq", bufs=2)
            ssq = sb.tile([P, 1], f32, tag="ssq", name="ssq", bufs=4)
            nc.scalar.activation(sq[:], xr[:], ACT.Square, accum_out=ssq[:])
            negmean = sb.tile([P, 1], f32, tag="negmean", name="negmean", bufs=4)
            nc.vector.tensor_reduce(negmean[:], xr[:], axis=AX.X, op=OP.add,
                                    negate=True)
            nc.vector.tensor_scalar_mul(negmean[:], negmean[:], 1.0 / D)
            var = sb.tile([P, 1], f32, tag="var", name="var", bufs=4)
            nc.vector.tensor_scalar_mul(var[:], ssq[:], 1.0 / D)
            msq = sb.tile([P, 1], f32, tag="msq", name="msq", bufs=4)
            nc.vector.tensor_tensor(msq[:], negmean[:], negmean[:], OP.mult)
            nc.vector.tensor_tensor(var[:], var[:], msq[:], OP.subtract)
            std = sb.tile([P, 1], f32, tag="std", name="std", bufs=4)
            nc.scalar.activation(std[:], var[:], ACT.Sqrt, bias=epsb[:])
            rstd = sb.tile([P, 1], f32, tag="rstd", name="rstd", bufs=4)
            nc.vector.reciprocal(rstd[:], std[:])
            y = sb.tile([P, D], f32, tag=out_tag, name=out_tag, bufs=bufs)
            nc.vector.tensor_scalar(y[:], xr[:], negmean[:], rstd[:],
                                    OP.add, OP.mult)
            if use_ln_gb:
                nc.vector.tensor_tensor(y[:], y[:], gw[:], OP.mult)
                nc.vector.tensor_tensor(y[:], y[:], bw[:], OP.add)
            return y

        # ---- stage 0: x = attn.T @ embg + pe ----
        x = [sb.tile([P, D], f32, tag="x", name="x", bufs=4) for _ in range(N_IB)]
        for ib in range(N_IB):
            ps_x = pp_big.tile([P, 512], f32, tag="mm", name="mm")
            nc.tensor.matmul(ps_x[:, :D], attn_sb[:, ib * P:(ib + 1) * P],
                             embg_sb[:], start=True, stop=True)
            nc.vector.tensor_tensor(x[ib][:], ps_x[:, :D], pe_sb[ib][:], OP.add)

        # ---- transformer layers ----
        for l in range(NL):
            wq_t = sb.tile([P, N_KB, D], f32r, tag="wq", name="wq", bufs=1)
            nc.sync.dma_start(wq_t[:], wq_d[l])
            wk_t = sb.tile([P, N_KB, D], f32r, tag="wk", name="wk", bufs=1)
            nc.sync.dma_start(wk_t[:], wk_d[l])
            wv_t = sb.tile([P, N_KB, D], f32r, tag="wv", name="wv", bufs=1)
            nc.sync.dma_start(wv_t[:], wv_d[l])
            wo_t = sb.tile([P, N_KB, D], f32r, tag="wo", name="wo", bufs=1)
            nc.sync.dma_start(wo_t[:], wo_d[l])
            b1_t = sb.tile([P, N_FB], f32, tag="b1", name="b1", bufs=2)
            nc.sync.dma_start(b1_t[:], b1_d[l])
            if use_ln_gb:
                ln_t = [sb.tile([P, D], f32, tag=f"lnw{j}", name=f"lnw{j}", bufs=2)
                        for j in range(4)]
                for j in range(4):
                    nc.sync.dma_start(ln_t[j][:], lnw_d[l, j])
            else:
                ln_t = [None] * 4
            if use_b2:
                b2_t = sb.tile([P, D], f32, tag="b2r", name="b2r", bufs=2)
                nc.sync.dma_start(b2_t[:], b2r_d[l])

            xT = transpose_into(x, "xT")

            # qT / kT full tiles [128 (d), 256 (seq)], q pre-scaled by 1/8;
            # halves also written into the persistent zero-padded qTp/kTp
            qT, kT = [], []
            for which, wt, dst, padded in (("q", wq_t, qT, qTp),
                                           ("k", wk_t, kT, kTp)):
                for nb in range(N_KB):
                    ps_q = pp_med.tile([P, LL], f32, tag="med", name="med")
                    for kb in range(N_KB):
                        nc.tensor.matmul(ps_q[:], wt[:, kb, nb * P:(nb + 1) * P],
                                         xT[kb][:],
                                         start=(kb == 0), stop=(kb == N_KB - 1))
                    t = sb.tile([P, LL], f32r, tag=f"{which}T{nb}",
                                name=f"{which}T{nb}", bufs=2)
                    scl = (1.0 / float(np.sqrt(DH))) if which == "q" else 1.0
                    nc.scalar.activation(t[:], ps_q[:], ACT.Copy, scale=scl)
                    for half in range(2):
                        h = 2 * nb + half
                        sl = slice(DH * half, DH * (half + 1))
                        nc.scalar.activation(padded[h][sl, :], ps_q[sl, :],
                                             ACT.Copy, scale=scl)
                    dst.append(t)

            # v in normal layout: [128 (seq j), 512 (d)] x2
            v = []
            for ib in range(N_IB):
                ps_v = pp_big.tile([P, 512], f32, tag="mm", name="mm")
                for kb in range(N_KB):
                    nc.tensor.matmul(ps_v[:, :D], xT[kb][:, ib * P:(ib + 1) * P],
                                     wv_t[:, kb, :],
                                     start=(kb == 0), stop=(kb == N_KB - 1))
                t = sb.tile([P, D], f32r, tag=f"v{ib}", name=f"v{ib}", bufs=2)
                nc.vector.tensor_copy(t[:], ps_v[:, :D])
                v.append(t)

            # attention: unnormalized exp; denominator folded into out-copy
            ps_o = [pp_o.tile([P, 512], f32, tag="po", name="po")
                    for _ in range(N_IB)]
            rinv = sb.tile([P, 2 * NH], f32, tag="rinv", name="rinv", bufs=2)
            for h in range(NH):
                ht = h // 2
                # s rows (i on partitions) -> row sums only
                for ib in range(N_IB):
                    ps_s = pp_med.tile([P, LL], f32, tag="med", name="med")
                    nc.tensor.matmul(ps_s[:], qTp[h][:, ib * P:(ib + 1) * P],
                                     kT[ht][:], start=True, stop=True)
                    if use_mask_bias:
                        nc.vector.tensor_tensor(ps_s[:], ps_s[:], maskb_sb[:],
                                                OP.add)
                    u = sb.tile([P, LL], f32, tag="u", name="u", bufs=2)
                    rs = sb.tile([P, 1], f32, tag="rs", name="rs", bufs=4)
                    nc.scalar.activation(u[:], ps_s[:], ACT.Exp, accum_out=rs[:])
                    nc.vector.reciprocal(rinv[:, 2 * h + ib:2 * h + ib + 1], rs[:])
                # sT (j on partitions) -> unnormalized attention, transposed
                uT = []
                for jb in range(N_IB):
                    ps_t = pp_med.tile([P, LL], f32, tag="med", name="med")
                    nc.tensor.matmul(ps_t[:], kTp[h][:, jb * P:(jb + 1) * P],
                                     qT[ht][:], start=True, stop=True)
                    if use_mask_bias:
                        nc.vector.tensor_scalar_add(ps_t[:], ps_t[:],
                                                    maskbT_sb[jb][:])
                    ut = sb.tile([P, LL], f32r, tag="uT", name="uT", bufs=3)
                    nc.scalar.activation(ut[:], ps_t[:], ACT.Exp)
                    uT.append(ut)
                # o_unnorm[i, d_h] += uT.T @ v_h
                for ib in range(N_IB):
                    for jb in range(N_IB):
                        nc.tensor.matmul(ps_o[ib][:, DH * h:DH * (h + 1)],
                                         uT[jb][:, ib * P:(ib + 1) * P],
                                         v[jb][:, DH * h:DH * (h + 1)],
                                         start=(jb == 0), stop=(jb == N_IB - 1))
            # normalize during psum -> sbuf copy (scale = 1/rowsum per head)
            o_sb = [sb.tile([P, D], f32, tag=f"o{ib}", name=f"o{ib}", bufs=2)
                    for ib in range(N_IB)]
            for ib in range(N_IB):
                for h in range(NH):
                    nc.scalar.activation(o_sb[ib][:, DH * h:DH * (h + 1)],
                                         ps_o[ib][:, DH * h:DH * (h + 1)],
                                         ACT.Copy,
                                         scale=rinv[:, 2 * h + ib:2 * h + ib + 1])
            oT = transpose_into(o_sb, "oT")

            # attn_out + residual -> LN1 -> x1
            x1 = []
            for ib in range(N_IB):
                ps_ao = pp_big.tile([P, 512], f32, tag="mm", name="mm")
                for kb in range(N_KB):
                    nc.tensor.matmul(ps_ao[:, :D], oT[kb][:, ib * P:(ib + 1) * P],
                                     wo_t[:, kb, :],
                                     start=(kb == 0), stop=(kb == N_KB - 1))
                x1.append(layer_norm(ps_ao[:, :D], x[ib], ln_t[0], ln_t[1],
                                     "x1", 2))

            x1T = transpose_into(x1, "x1T")

            # FFN fused: per fb compute hT[fb], then accumulate into ffn psums
            ps_f = [pp_big.tile([P, 512], f32, tag="mm", name="mm")
                    for _ in range(N_IB)]
            for fb in range(N_FB):
                w1_t = sb.tile([P, N_KB, P], f32r, tag="w1t", name="w1t", bufs=4)
                nc.sync.dma_start(w1_t[:], w1_d[l, fb])
                w2_t = sb.tile([P, D], f32r, tag="w2t", name="w2t", bufs=4)
                nc.sync.dma_start(w2_t[:], w2_d[l, fb])
                ps_h = pp_med.tile([P, LL], f32, tag="med", name="med")
                for kb in range(N_KB):
                    nc.tensor.matmul(ps_h[:], w1_t[:, kb, :], x1T[kb][:],
                                     start=(kb == 0), stop=(kb == N_KB - 1))
                hT = sb.tile([P, LL], f32r, tag="hT", name="hT", bufs=3)
                nc.scalar.activation(hT[:], ps_h[:], ACT.Relu,
                                     bias=b1_t[:, fb:fb + 1])
                for ib in range(N_IB):
                    nc.tensor.matmul(ps_f[ib][:, :D], hT[:, ib * P:(ib + 1) * P],
                                     w2_t[:],
                                     start=(fb == 0), stop=(fb == N_FB - 1))

            x_next = []
            for ib in range(N_IB):
                if use_b2:
                    nc.vector.tensor_tensor(ps_f[ib][:, :D], ps_f[ib][:, :D],
                                            b2_t[:], OP.add)
                x_next.append(layer_norm(ps_f[ib][:, :D], x1[ib], ln_t[2],
                                         ln_t[3], "x", 4))
            x = x_next

        # ---- final projection: out = x @ projT ----
        xTf = transpose_into(x, "xT")
        for pb in range(N_PB):
            pj_t = sb.tile([P, N_KB, PBW], f32r, tag="pjt", name="pjt", bufs=3)
            nc.sync.dma_start(pj_t[:], projr_d[pb])
            for ib in range(N_IB):
                ps_p = pp_big.tile([P, 512], f32, tag="mm", name="mm")
                for kb in range(N_KB):
                    nc.tensor.matmul(ps_p[:, :PBW], xTf[kb][:, ib * P:(ib + 1) * P],
                                     pj_t[:, kb, :],
                                     start=(kb == 0), stop=(kb == N_KB - 1))
                o_t = sb.tile([P, PBW], f32, tag="outsb", name="outsb", bufs=4)
                if (pb + ib) % 2 == 0:
                    nc.scalar.copy(o_t[:], ps_p[:, :PBW])
                else:
                    nc.vector.tensor_copy(o_t[:], ps_p[:, :PBW])
                nc.scalar.dma_start(out_d[ib * P:(ib + 1) * P,
                                          pb * PBW:(pb + 1) * PBW], o_t[:])

    _split_excess_waits(nc, mybir, maxw=1)
    return nc


def _split_excess_waits(nc, mybir, maxw=1):
    """This container's walrus rejects instructions with more than `maxw`
    sync waits.  Move excess immediate sem waits onto standalone EVSEM
    instructions inserted just before, on the same engine."""
    for fn in nc.m.functions:
        for blk in fn.blocks:
            out = []
            changed = False
            for ins in blk.instructions:
                si = ins.sync_info
                if si is not None and len(si.on_wait) > maxw:
                    waits = list(si.on_wait)
                    movable = [w for w in waits
                               if w.sync_type == "semaphore" and w.wait_reg is None]
                    fixed = [w for w in waits if w not in movable]
                    keep_budget = maxw - len(fixed)
                    assert keep_budget >= 0, f"unmovable waits exceed limit: {ins}"
                    keep = movable[len(movable) - keep_budget:] if keep_budget else []
                    move = movable[:len(movable) - keep_budget]
                    for i in range(0, len(move), maxw):
                        ev = mybir.InstEventSemaphore(
                            name=f"I-{nc.next_id()}", engine=ins.engine)
                        ev.sync_info = mybir.SyncInfo(
                            on_wait=move[i:i + maxw], on_update=[])
                        nc.register_instruction(ev, overwrite=True)
                        out.append(ev)
                    ins.sync_info = mybir.SyncInfo(
                        on_wait=fixed + keep, on_update=list(si.on_update))
                    changed = True
                out.append(ins)
            if changed:
                blk.instructions = out


def _host_prepare(inputs):
    """Returns (shared_map, per_core_list, flags)."""
    g = {k: np.asarray(v) for k, v in inputs.items()}
    attn, mask, tgt = g["attn"], g["mask"], g["tgt"]
    emb_w, proj_w = np.asarray(g["emb_w"], F32), np.asarray(g["proj_w"], F32)

    # positional encoding (matches reference)
    pos = np.arange(LL, dtype=F32)[:, None]
    div = np.exp(np.arange(0, D, 2, dtype=F32) * (-np.log(10000.0) / D))
    pe = np.zeros((LL, D), F32)
    pe[:, 0::2] = np.sin(pos * div)
    pe[:, 1::2] = np.cos(pos * div)
    pe_r = np.ascontiguousarray(pe.reshape(N_IB, P, D))

    def kmajor(w):  # (512, N) -> (128, 4, N)
        n = w.shape[1]
        return np.ascontiguousarray(
            np.asarray(w, F32).reshape(N_KB, P, n).transpose(1, 0, 2))

    wq = np.stack([kmajor(g["Wq"][l]) for l in range(NL)])
    wk = np.stack([kmajor(g["Wk"][l]) for l in range(NL)])
    wv = np.stack([kmajor(g["Wv"][l]) for l in range(NL)])
    wo = np.stack([kmajor(g["Wo"][l]) for l in range(NL)])
    # W1: (512, 2048) -> (16, 128, 4, 128)
    w1 = np.stack([
        np.ascontiguousarray(
            np.asarray(g["W1"][l], F32).reshape(N_KB, P, N_FB, P)
            .transpose(2, 1, 0, 3))
        for l in range(NL)])
    # W2: (2048, 512) -> (16, 128, 512)
    w2 = np.stack([
        np.ascontiguousarray(np.asarray(g["W2"][l], F32).reshape(N_FB, P, D))
        for l in range(NL)])
    b1 = np.stack([
        np.ascontiguousarray(np.asarray(g["b1"][l], F32).reshape(N_FB, P).T)
        for l in range(NL)])
    projr = np.ascontiguousarray(
        proj_w.T.reshape(N_KB, P, N_PB, PBW).transpose(2, 1, 0, 3))

    ln_vecs = [np.asarray(g[k], F32) for k in ("ln1_g", "ln1_b", "ln2_g", "ln2_b")]
    use_ln_gb = not (np.all(ln_vecs[0] == 1) and np.all(ln_vecs[1] == 0)
                     and np.all(ln_vecs[2] == 1) and np.all(ln_vecs[3] == 0))
    use_b2 = bool(np.any(np.asarray(g["b2"], F32) != 0))
    use_mask_bias = not bool(np.asarray(mask).all())

    ones = np.ones((P, 1), F32)
    shared = dict(pe=pe_r, wq=wq, wk=wk, wv=wv, wo=wo, w1=w1, w2=w2, b1=b1,
                  projr=projr)
    if use_ln_gb:
        shared["lnw"] = np.stack([
            np.stack([ones * v[l][None, :] for v in ln_vecs])
            for l in range(NL)])
    if use_b2:
        shared["b2r"] = np.stack([ones * np.asarray(g["b2"][l], F32)[None, :]
                                  for l in range(NL)])

    per_core = []
    sqrt_d = np.sqrt(np.float32(D))
    for b in range(BS):
        tg = np.asarray(tgt[b]).astype(np.int64)
        embg = emb_w[tg] * (tg != 0)[:, None].astype(F32) * sqrt_d
        m = dict(attn=np.ascontiguousarray(np.asarray(attn[b], F32)),
                 embg=np.ascontiguousarray(embg.astype(F32)))
        if use_mask_bias:
            mb = np.where(np.asarray(mask[b]), 0.0, -1e9).astype(F32)
            m["maskb"] = np.ascontiguousarray(ones * mb[None, :])
            m["maskbT"] = np.ascontiguousarray(mb.reshape(N_IB, P, 1))
        per_core.append(m)
    return shared, per_core, (use_mask_bias, use_ln_gb, use_b2)


def kernel(**inputs):
    from concourse import bass_utils

    shared, per_core, flags = _host_prepare(inputs)
    nc = _build_bass(*flags)
    in_maps = [{**shared, **pc} for pc in per_core]
    res = bass_utils.run_bass_kernel_spmd(nc, in_maps, core_ids=list(range(BS)),
                                          trace=False)
    out = np.stack([res.results[b]["out"] for b in range(BS)])
    return out.astype(F32)


# revision 12
# speedup vs baseline: 1.2113x; 1.2113x over previous
"""Trainium2 Bass kernel for nn_AttnAligner.

Strategy: data-parallel over the batch (8 samples -> 8 NeuronCores), one
sample per core.  The (bs, ps, ll) scatter matrix is never materialized:

    alg.T @ emb_w  ==  attn[b].T @ emb_w[tgt[b]]        (segment_sum is linear)

so the host gathers the 64 embedding rows per sample (zeroing rows where
tgt == 0, which reproduces the `alg[:, :, 0] = 0` vocab-0 fill), and the
device runs: tiny matmul -> 4-layer transformer encoder -> (256, 16000)
output projection.  proj_w is transposed on the host so the device only
does k-major streaming matmuls.  Matmuls use float32r (full-rate fp32)
with K padded to 128; attention uses the unnormalized-exp trick (softmax
denominator folded into the output-copy scale) and computes both s and
s^T on the PE so no per-head transposes are needed.
"""

import numpy as np

BS, LS, LL = 8, 64, 256
PS, D, NL, NH = 16000, 512, 4, 8
DH = D // NH          # 64
DF = 4 * D            # 2048
P = 128
N_IB = LL // P        # 2   seq-partition blocks
N_KB = D // P         # 4   d-partition blocks
N_FB = DF // P        # 16  ffn-partition blocks
PBW = 500             # proj column chunk (<=512 psum free dim)
N_PB = PS // PBW      # 32
F32 = np.float32
USE_BF16 = True


def _build_bass(use_mask_bias: bool, use_ln_gb: bool, use_b2: bool):
    import concourse.bass as bass
    import concourse.mybir as mybir
    import concourse.tile as tile
    from concourse.masks import make_identity
    from contextlib import ExitStack
    from bass_rust import ScopedClock

    f32 = mybir.dt.float32
    f32r = mybir.dt.float32r
    mmdt = mybir.dt.bfloat16 if USE_BF16 else f32r
    AX = mybir.AxisListType
    OP = mybir.AluOpType
    ACT = mybir.ActivationFunctionType

    class PatchedTC(tile.TileContext):
        """The walrus build in this container rejects >2 sync waits on the
        kernel-tail Drain.  Emit the waits as individual EVSEM wait
        instructions instead, then a waitless drain."""

        def _drain_and_barrier(self, tick_clock, wait_clock):
            dummy = mybir.InstDrain(
                name=f"I-{self.nc.next_id()}", engine=mybir.EngineType.SP
            )
            wait_clock.add_sem_waits(
                dummy, ScopedClock({None: tick_clock.global_clock})
            )
            waits = dummy.sync_info.on_wait if dummy.sync_info is not None else []
            assert self.sems is not None
            handles = {h.name: h for h in self.sems.allocated().values()}
            for w in waits:
                self.nc.sync.wait_ge(handles[w.ant_name], w.wait_value)
            self.nc.sync.drain()
            self.nc.all_engine_barrier()
            popped = self.nc._tile_sem_poison_stack.pop()
            assert popped is self._sem_poison
            self.nc.clear_and_free_semaphores(list(self.sems.allocated().values()))
            self.nc.all_engine_barrier()

    nc = bass.Bass("TRN2", target_bir_lowering=False, debug=False)

    def din(name, shape, dt=None):
        return nc.dram_tensor(name, list(shape), dt or f32,
                              kind="ExternalInput").ap()

    attn_d = din("attn", (LS, LL), f32r)
    embg_d = din("embg", (LS, D), f32r)
    pe_d = din("pe", (N_IB, P, D))
    wq_d = din("wq", (NL, P, N_KB, D), mmdt)
    wk_d = din("wk", (NL, P, N_KB, D), mmdt)
    wv_d = din("wv", (NL, P, N_KB, D), mmdt)
    wo_d = din("wo", (NL, P, N_KB, D), mmdt)
    w1_d = din("w1", (NL, N_FB, P, N_KB, P), mmdt)
    w2_d = din("w2", (NL, N_FB, P, D), mmdt)
    b1_d = din("b1", (NL, P, N_FB))
    if use_ln_gb:
        lnw_d = din("lnw", (NL, 4, P, D))   # ln1_g, ln1_b, ln2_g, ln2_b replicated
    if use_b2:
        b2r_d = din("b2r", (NL, P, D))
    projr_d = din("projr", (N_PB, P, N_KB, PBW), mmdt)
    if use_mask_bias:
        maskb_d = din("maskb", (P, LL))      # (1-mask)*-1e9 replicated over rows
        maskbT_d = din("maskbT", (N_IB, P, 1))
    out_d = nc.dram_tensor("out", [LL, PS], f32, kind="ExternalOutput").ap()

    with PatchedTC(nc) as tc, ExitStack() as stk:
        const = stk.enter_context(tc.tile_pool(name="const", bufs=1))
        sb = stk.enter_context(tc.tile_pool(name="sb", bufs=2))
        pp_big = stk.enter_context(tc.tile_pool(name="pp_big", bufs=2, space="PSUM"))
        pp_med = stk.enter_context(tc.tile_pool(name="pp_med", bufs=2, space="PSUM"))
        pp_o = stk.enter_context(tc.tile_pool(name="pp_o", bufs=2, space="PSUM"))
        pp_tr = stk.enter_context(tc.tile_pool(name="pp_tr", bufs=2, space="PSUM"))

        ident = const.tile([P, P], f32, tag="ident", name="ident")
        make_identity(nc, ident)
        epsb = const.tile([P, 1], f32, tag="epsb", name="epsb")
        nc.vector.memset(epsb[:], 1e-5)

        # persistent zero-padded per-head stationaries: rows of the "other"
        # head stay zero forever, only the live half is rewritten per layer
        qTp = [const.tile([P, LL], mmdt, tag=f"qTp{h}", name=f"qTp{h}")
               for h in range(NH)]
        kTp = [const.tile([P, LL], mmdt, tag=f"kTp{h}", name=f"kTp{h}")
               for h in range(NH)]
        for h in range(NH):
            if USE_BF16:
                nc.vector.memset(qTp[h][:], 0.0)
                nc.vector.memset(kTp[h][:], 0.0)
            else:
                nc.vector.memset(qTp[h][:].bitcast(f32), 0.0)
                nc.vector.memset(kTp[h][:].bitcast(f32), 0.0)

        pe_sb = [const.tile([P, D], f32, tag=f"pe{ib}", name=f"pe{ib}")
                 for ib in range(N_IB)]
        for ib in range(N_IB):
            nc.sync.dma_start(pe_sb[ib][:], pe_d[ib])
        attn_sb = const.tile([LS, LL], f32r, tag="attn", name="attn")
        nc.sync.dma_start(attn_sb[:], attn_d[:])
        embg_sb = const.tile([LS, D], f32r, tag="embg", name="embg")
        nc.sync.dma_start(embg_sb[:], embg_d[:])
        if use_mask_bias:
            maskb_sb = const.tile([P, LL], f32, tag="maskb", name="maskb")
            nc.sync.dma_start(maskb_sb[:], maskb_d[:])
            maskbT_sb = [const.tile([P, 1], f32, tag=f"maskbT{j}", name=f"maskbT{j}")
                         for j in range(N_IB)]
            for j in range(N_IB):
                nc.sync.dma_start(maskbT_sb[j][:], maskbT_d[j])

        def transpose_into(src_tiles, tag):
            """src: list of 2 [128, 512] seq-major f32 tiles -> 4 [128, 256]
            d-major f32r tiles."""
            dst = [sb.tile([P, LL], mmdt, tag=f"{tag}{kb}", name=f"{tag}{kb}",
                           bufs=2) for kb in range(N_KB)]
            for kb in range(N_KB):
                for ib in range(N_IB):
                    pt = pp_tr.tile([P, P], f32, tag="tr", name="tr")
                    nc.tensor.transpose(pt[:], src_tiles[ib][:, kb * P:(kb + 1) * P],
                                        ident[:])
                    nc.vector.tensor_copy(dst[kb][:, ib * P:(ib + 1) * P], pt[:])
            return dst

        def layer_norm(ps_in, resid, gw, bw, out_tag, bufs):
            """y = LN(psum + resid); var via E[x^2] - mu^2."""
            xr = sb.tile([P, D], f32, tag=f"{out_tag}_pre", name=f"{out_tag}_pre",
                         bufs=2)
            nc.vector.tensor_tensor(xr[:], ps_in[:], resid[:], OP.add)
            sq = sb.tile([P, D], f32, tag="sq", name="sq", bufs=2)
            ssq = sb.tile([P, 1], f32, tag="ssq", name="ssq", bufs=4)
            nc.scalar.activation(sq[:], xr[:], ACT.Square, accum_out=ssq[:])
            negmean = sb.tile([P, 1], f32, tag="negmean", name="negmean", bufs=4)
            nc.vector.tensor_reduce(negmean[:], xr[:], axis=AX.X, op=OP.add,
                                    negate=True)
            nc.vector.tensor_scalar_mul(negmean[:], negmean[:], 1.0 / D)
            var = sb.tile([P, 1], f32, tag="var", name="var", bufs=4)
            nc.vector.tensor_scalar_mul(var[:], ssq[:], 1.0 / D)
            msq = sb.tile([P, 1], f32, tag="msq", name="msq", bufs=4)
            nc.vector.tensor_tensor(msq[:], negmean[:], negmean[:], OP.mult)
            nc.vector.tensor_tensor(var[:], var[:], msq[:], OP.subtract)
            std = sb.tile([P, 1], f32, tag="std", name="std", bufs=4)
            nc.scalar.activation(std[:], var[:], ACT.Sqrt, bias=epsb[:])
            rstd = sb.tile([P, 1], f32, tag="rstd", name="rstd", bufs=4)
            nc.vector.reciprocal(rstd[:], std[:])
            y = sb.tile([P, D], f32, tag=out_tag, name=out_tag, bufs=bufs)
            nc.vector.tensor_scalar(y[:], xr[:], negmean[:], rstd[:],
                                    OP.add, OP.mult)
            if use_ln_gb:
                nc.vector.tensor_tensor(y[:], y[:], gw[:], OP.mult)
                nc.vector.tensor_tensor(y[:], y[:], bw[:], OP.add)
            return y

        # ---- stage 0: x = attn.T @ embg + pe ----
        x = [sb.tile([P, D], f32, tag="x", name="x", bufs=4) for _ in range(N_IB)]
        for ib in range(N_IB):
            ps_x = pp_big.tile([P, 512], f32, tag="mm", name="mm")
            nc.tensor.matmul(ps_x[:, :D], attn_sb[:, ib * P:(ib + 1) * P],
                             embg_sb[:], start=True, stop=True)
            nc.vector.tensor_tensor(x[ib][:], ps_x[:, :D], pe_sb[ib][:], OP.add)

        # ---- transformer layers ----
        for l in range(NL):
            wq_t = sb.tile([P, N_KB, D], mmdt, tag="wq", name="wq", bufs=1)
            nc.sync.dma_start(wq_t[:], wq_d[l])
            wk_t = sb.tile([P, N_KB, D], mmdt, tag="wk", name="wk", bufs=1)
            nc.sync.dma_start(wk_t[:], wk_d[l])
            wv_t = sb.tile([P, N_KB, D], mmdt, tag="wv", name="wv", bufs=1)
            nc.sync.dma_start(wv_t[:], wv_d[l])
            wo_t = sb.tile([P, N_KB, D], mmdt, tag="wo", name="wo", bufs=1)
            nc.sync.dma_start(wo_t[:], wo_d[l])
            b1_t = sb.tile([P, N_FB], f32, tag="b1", name="b1", bufs=2)
            nc.sync.dma_start(b1_t[:], b1_d[l])
            if use_ln_gb:
                ln_t = [sb.tile([P, D], f32, tag=f"lnw{j}", name=f"lnw{j}", bufs=2)
                        for j in range(4)]
                for j in range(4):
                    nc.sync.dma_start(ln_t[j][:], lnw_d[l, j])
            else:
                ln_t = [None] * 4
            if use_b2:
                b2_t = sb.tile([P, D], f32, tag="b2r", name="b2r", bufs=2)
                nc.sync.dma_start(b2_t[:], b2r_d[l])

            xT = transpose_into(x, "xT")

            # qT / kT full tiles [128 (d), 256 (seq)], q pre-scaled by 1/8;
            # halves also written into the persistent zero-padded qTp/kTp
            qT, kT = [], []
            for which, wt, dst, padded in (("q", wq_t, qT, qTp),
                                           ("k", wk_t, kT, kTp)):
                for nb in range(N_KB):
                    ps_q = pp_med.tile([P, LL], f32, tag="med", name="med")
                    for kb in range(N_KB):
                        nc.tensor.matmul(ps_q[:], wt[:, kb, nb * P:(nb + 1) * P],
                                         xT[kb][:],
                                         start=(kb == 0), stop=(kb == N_KB - 1))
                    t = sb.tile([P, LL], mmdt, tag=f"{which}T{nb}",
                                name=f"{which}T{nb}", bufs=2)
                    if which == "q":
                        scl = 1.0 / float(np.sqrt(DH))
                        nc.scalar.activation(t[:], ps_q[:], ACT.Copy, scale=scl)
                        for half in range(2):
                            h = 2 * nb + half
                            sl = slice(DH * half, DH * (half + 1))
                            nc.scalar.activation(padded[h][sl, :], ps_q[sl, :],
                                                 ACT.Copy, scale=scl)
                    else:
                        nc.vector.tensor_copy(t[:], ps_q[:])
                        for half in range(2):
                            h = 2 * nb + half
                            sl = slice(DH * half, DH * (half + 1))
                            nc.vector.tensor_copy(padded[h][sl, :], ps_q[sl, :])
                    dst.append(t)

            # v in normal layout: [128 (seq j), 512 (d)] x2
            v = []
            for ib in range(N_IB):
                ps_v = pp_big.tile([P, 512], f32, tag="mm", name="mm")
                for kb in range(N_KB):
                    nc.tensor.matmul(ps_v[:, :D], xT[kb][:, ib * P:(ib + 1) * P],
                                     wv_t[:, kb, :],
                                     start=(kb == 0), stop=(kb == N_KB - 1))
                t = sb.tile([P, D], mmdt, tag=f"v{ib}", name=f"v{ib}", bufs=2)
                nc.vector.tensor_copy(t[:], ps_v[:, :D])
                v.append(t)

            # attention: unnormalized exp; denominator folded into out-copy
            ps_o = [pp_o.tile([P, 512], f32, tag="po", name="po")
                    for _ in range(N_IB)]
            rinv = sb.tile([P, 2 * NH], f32, tag="rinv", name="rinv", bufs=2)
            for h in range(NH):
                ht = h // 2
                # s rows (i on partitions) -> row sums only
                for ib in range(N_IB):
                    ps_s = pp_med.tile([P, LL], f32, tag="med", name="med")
                    nc.tensor.matmul(ps_s[:], qTp[h][:, ib * P:(ib + 1) * P],
                                     kT[ht][:], start=True, stop=True)
                    if use_mask_bias:
                        nc.vector.tensor_tensor(ps_s[:], ps_s[:], maskb_sb[:],
                                                OP.add)
                    u = sb.tile([P, LL], f32, tag="u", name="u", bufs=2)
                    rs = sb.tile([P, 1], f32, tag="rs", name="rs", bufs=4)
                    nc.scalar.activation(u[:], ps_s[:], ACT.Exp, accum_out=rs[:])
                    nc.vector.reciprocal(rinv[:, 2 * h + ib:2 * h + ib + 1], rs[:])
                # sT (j on partitions) -> unnormalized attention, transposed
                uT = []
                for jb in range(N_IB):
                    ps_t = pp_med.tile([P, LL], f32, tag="med", name="med")
                    nc.tensor.matmul(ps_t[:], kTp[h][:, jb * P:(jb + 1) * P],
                                     qT[ht][:], start=True, stop=True)
                    if use_mask_bias:
                        nc.vector.tensor_scalar_add(ps_t[:], ps_t[:],
                                                    maskbT_sb[jb][:])
                    ut = sb.tile([P, LL], mmdt, tag="uT", name="uT", bufs=3)
                    nc.scalar.activation(ut[:], ps_t[:], ACT.Exp)
                    uT.append(ut)
                # o_unnorm[i, d_h] += uT.T @ v_h
                for ib in range(N_IB):
                    for jb in range(N_IB):
                        nc.tensor.matmul(ps_o[ib][:, DH * h:DH * (h + 1)],
                                         uT[jb][:, ib * P:(ib + 1) * P],
                                         v[jb][:, DH * h:DH * (h + 1)],
                                         start=(jb == 0), stop=(jb == N_IB - 1))
            # normalize during psum -> sbuf copy (scale = 1/rowsum per head)
            o_sb = [sb.tile([P, D], f32, tag=f"o{ib}", name=f"o{ib}", bufs=2)
                    for ib in range(N_IB)]
            for ib in range(N_IB):
                for h in range(NH):
                    nc.scalar.activation(o_sb[ib][:, DH * h:DH * (h + 1)],
                                         ps_o[ib][:, DH * h:DH * (h + 1)],
                                         ACT.Copy,
                                         scale=rinv[:, 2 * h + ib:2 * h + ib + 1])
            oT = transpose_into(o_sb, "oT")

            # attn_out + residual -> LN1 -> x1
            x1 = []
            for ib in range(N_IB):
                ps_ao = pp_big.tile([P, 512], f32, tag="mm", name="mm")
                for kb in range(N_KB):
                    nc.tensor.matmul(ps_ao[:, :D], oT[kb][:, ib * P:(ib + 1) * P],
                                     wo_t[:, kb, :],
                                     start=(kb == 0), stop=(kb == N_KB - 1))
                x1.append(layer_norm(ps_ao[:, :D], x[ib], ln_t[0], ln_t[1],
                                     "x1", 2))

            x1T = transpose_into(x1, "x1T")

            # FFN fused: per fb compute hT[fb], then accumulate into ffn psums
            ps_f = [pp_big.tile([P, 512], f32, tag="mm", name="mm")
                    for _ in range(N_IB)]
            for fb in range(N_FB):
                w1_t = sb.tile([P, N_KB, P], mmdt, tag="w1t", name="w1t", bufs=4)
                nc.sync.dma_start(w1_t[:], w1_d[l, fb])
                w2_t = sb.tile([P, D], mmdt, tag="w2t", name="w2t", bufs=4)
                nc.sync.dma_start(w2_t[:], w2_d[l, fb])
                ps_h = pp_med.tile([P, LL], f32, tag="med", name="med")
                for kb in range(N_KB):
                    nc.tensor.matmul(ps_h[:], w1_t[:, kb, :], x1T[kb][:],
                                     start=(kb == 0), stop=(kb == N_KB - 1))
                hT = sb.tile([P, LL], mmdt, tag="hT", name="hT", bufs=3)
                nc.scalar.activation(hT[:], ps_h[:], ACT.Relu,
                                     bias=b1_t[:, fb:fb + 1])
                for ib in range(N_IB):
                    nc.tensor.matmul(ps_f[ib][:, :D], hT[:, ib * P:(ib + 1) * P],
                                     w2_t[:],
                                     start=(fb == 0), stop=(fb == N_FB - 1))

            x_next = []
            for ib in range(N_IB):
                if use_b2:
                    nc.vector.tensor_tensor(ps_f[ib][:, :D], ps_f[ib][:, :D],
                                            b2_t[:], OP.add)
                x_next.append(layer_norm(ps_f[ib][:, :D], x1[ib], ln_t[2],
                                         ln_t[3], "x", 4))
            x = x_next

        # ---- final projection: out = x @ projT ----
        xTf = transpose_into(x, "xT")
        for pb in range(N_PB):
            pj_t = sb.tile([P, N_KB, PBW], mmdt, tag="pjt", name="pjt", bufs=3)
            nc.sync.dma_start(pj_t[:], projr_d[pb])
            for ib in range(N_IB):
                ps_p = pp_big.tile([P, 512], f32, tag="mm", name="mm")
                for kb in range(N_KB):
                    nc.tensor.matmul(ps_p[:, :PBW], xTf[kb][:, ib * P:(ib + 1) * P],
                                     pj_t[:, kb, :],
                                     start=(kb == 0), stop=(kb == N_KB - 1))
                o_t = sb.tile([P, PBW], f32, tag="outsb", name="outsb", bufs=4)
                if (pb + ib) % 2 == 0:
                    nc.scalar.copy(o_t[:], ps_p[:, :PBW])
                else:
                    nc.vector.tensor_copy(o_t[:], ps_p[:, :PBW])
                nc.sync.dma_start(out_d[ib * P:(ib + 1) * P,
                                          pb * PBW:(pb + 1) * PBW], o_t[:])

    _split_excess_waits(nc, mybir, maxw=1)
    return nc


def _split_excess_waits(nc, mybir, maxw=1):
    """This container's walrus rejects instructions with more than `maxw`
    sync waits.  Move excess immediate sem waits onto standalone EVSEM
    instructions inserted just before, on the same engine."""
    for fn in nc.m.functions:
        for blk in fn.blocks:
            out = []
            changed = False
            for ins in blk.instructions:
                si = ins.sync_info
                if si is not None and len(si.on_wait) > maxw:
                    waits = list(si.on_wait)
                    movable = [w for w in waits
                               if w.sync_type == "semaphore" and w.wait_reg is None]
                    fixed = [w for w in waits if w not in movable]
                    keep_budget = maxw - len(fixed)
                    assert keep_budget >= 0, f"unmovable waits exceed limit: {ins}"
                    keep = movable[len(movable) - keep_budget:] if keep_budget else []
                    move = movable[:len(movable) - keep_budget]
                    for i in range(0, len(move), maxw):
                        ev = mybir.InstEventSemaphore(
                            name=f"I-{nc.next_id()}", engine=ins.engine)
                        ev.sync_info = mybir.SyncInfo(
                            on_wait=move[i:i + maxw], on_update=[])
                        nc.register_instruction(ev, overwrite=True)
                        out.append(ev)
                    ins.sync_info = mybir.SyncInfo(
                        on_wait=fixed + keep, on_update=list(si.on_update))
                    changed = True
                out.append(ins)
            if changed:
                blk.instructions = out


def _host_prepare(inputs):
    """Returns (shared_map, per_core_list, flags)."""
    g = {k: np.asarray(v) for k, v in inputs.items()}
    attn, mask, tgt = g["attn"], g["mask"], g["tgt"]
    emb_w, proj_w = np.asarray(g["emb_w"], F32), np.asarray(g["proj_w"], F32)

    # positional encoding (matches reference)
    pos = np.arange(LL, dtype=F32)[:, None]
    div = np.exp(np.arange(0, D, 2, dtype=F32) * (-np.log(10000.0) / D))
    pe = np.zeros((LL, D), F32)
    pe[:, 0::2] = np.sin(pos * div)
    pe[:, 1::2] = np.cos(pos * div)
    pe_r = np.ascontiguousarray(pe.reshape(N_IB, P, D))

    def kmajor(w):  # (512, N) -> (128, 4, N)
        n = w.shape[1]
        return np.ascontiguousarray(
            np.asarray(w, F32).reshape(N_KB, P, n).transpose(1, 0, 2))

    import ml_dtypes
    mmnp = ml_dtypes.bfloat16 if USE_BF16 else F32

    wq = np.stack([kmajor(g["Wq"][l]) for l in range(NL)])
    wk = np.stack([kmajor(g["Wk"][l]) for l in range(NL)])
    wv = np.stack([kmajor(g["Wv"][l]) for l in range(NL)])
    wo = np.stack([kmajor(g["Wo"][l]) for l in range(NL)])
    # W1: (512, 2048) -> (16, 128, 4, 128)
    w1 = np.stack([
        np.ascontiguousarray(
            np.asarray(g["W1"][l], F32).reshape(N_KB, P, N_FB, P)
            .transpose(2, 1, 0, 3))
        for l in range(NL)])
    # W2: (2048, 512) -> (16, 128, 512)
    w2 = np.stack([
        np.ascontiguousarray(np.asarray(g["W2"][l], F32).reshape(N_FB, P, D))
        for l in range(NL)])
    b1 = np.stack([
        np.ascontiguousarray(np.asarray(g["b1"][l], F32).reshape(N_FB, P).T)
        for l in range(NL)])
    projr = np.ascontiguousarray(
        proj_w.T.reshape(N_KB, P, N_PB, PBW).transpose(2, 1, 0, 3))

    ln_vecs = [np.asarray(g[k], F32) for k in ("ln1_g", "ln1_b", "ln2_g", "ln2_b")]
    use_ln_gb = not (np.all(ln_vecs[0] == 1) and np.all(ln_vecs[1] == 0)
                     and np.all(ln_vecs[2] == 1) and np.all(ln_vecs[3] == 0))
    use_b2 = bool(np.any(np.asarray(g["b2"], F32) != 0))
    use_mask_bias = not bool(np.asarray(mask).all())

    ones = np.ones((P, 1), F32)
    shared = dict(pe=pe_r, b1=b1,
                  wq=wq.astype(mmnp), wk=wk.astype(mmnp), wv=wv.astype(mmnp),
                  wo=wo.astype(mmnp), w1=w1.astype(mmnp), w2=w2.astype(mmnp),
                  projr=projr.astype(mmnp))
    if use_ln_gb:
        shared["lnw"] = np.stack([
            np.stack([ones * v[l][None, :] for v in ln_vecs])
            for l in range(NL)])
    if use_b2:
        shared["b2r"] = np.stack([ones * np.asarray(g["b2"][l], F32)[None, :]
                                  for l in range(NL)])

    per_core = []
    sqrt_d = np.sqrt(np.float32(D))
    for b in range(BS):
        tg = np.asarray(tgt[b]).astype(np.int64)
        embg = emb_w[tg] * (tg != 0)[:, None].astype(F32) * sqrt_d
        m = dict(attn=np.ascontiguousarray(np.asarray(attn[b], F32)),
                 embg=np.ascontiguousarray(embg.astype(F32)))
        if use_mask_bias:
            mb = np.where(np.asarray(mask[b]), 0.0, -1e9).astype(F32)
            m["maskb"] = np.ascontiguousarray(ones * mb[None, :])
            m["maskbT"] = np.ascontiguousarray(mb.reshape(N_IB, P, 1))
        per_core.append(m)
    return shared, per_core, (use_mask_bias, use_ln_gb, use_b2)


def kernel(**inputs):
    from concourse import bass_utils

    shared, per_core, flags = _host_prepare(inputs)
    nc = _build_bass(*flags)
    in_maps = [{**shared, **pc} for pc in per_core]
    res = bass_utils.run_bass_kernel_spmd(nc, in_maps, core_ids=list(range(BS)),
                                          trace=False)
    out = np.stack([res.results[b]["out"] for b in range(BS)])
    return out.astype(F32)


# revision 14
# speedup vs baseline: 1.4291x; 1.1797x over previous
"""Trainium2 Bass kernel for nn_AttnAligner.

Strategy: data-parallel over the batch (8 samples -> 8 NeuronCores), one
sample per core.  The (bs, ps, ll) scatter matrix is never materialized:

    alg.T @ emb_w  ==  attn[b].T @ emb_w[tgt[b]]        (segment_sum is linear)

so the host gathers the 64 embedding rows per sample (zeroing rows where
tgt == 0, which reproduces the `alg[:, :, 0] = 0` vocab-0 fill), and the
device runs: tiny matmul -> 4-layer transformer encoder -> (256, 16000)
output projection.  proj_w is transposed on the host so the device only
does k-major streaming matmuls.  Matmuls use float32r (full-rate fp32)
with K padded to 128; attention uses the unnormalized-exp trick (softmax
denominator folded into the output-copy scale) and computes both s and
s^T on the PE so no per-head transposes are needed.
"""

import numpy as np

BS, LS, LL = 8, 64, 256
PS, D, NL, NH = 16000, 512, 4, 8
DH = D // NH          # 64
DF = 4 * D            # 2048
P = 128
N_IB = LL // P        # 2   seq-partition blocks
N_KB = D // P         # 4   d-partition blocks
N_FB = DF // P        # 16  ffn-partition blocks
PBW = 500             # proj column chunk (<=512 psum free dim)
N_PB = PS // PBW      # 32
F32 = np.float32
USE_BF16 = True


def _build_bass(use_mask_bias: bool, use_ln_gb: bool, use_b2: bool):
    import concourse.bass as bass
    import concourse.mybir as mybir
    import concourse.tile as tile
    from concourse.masks import make_identity
    from contextlib import ExitStack
    from bass_rust import ScopedClock

    f32 = mybir.dt.float32
    f32r = mybir.dt.float32r
    mmdt = mybir.dt.bfloat16 if USE_BF16 else f32r
    AX = mybir.AxisListType
    OP = mybir.AluOpType
    ACT = mybir.ActivationFunctionType

    class PatchedTC(tile.TileContext):
        """The walrus build in this container rejects >2 sync waits on the
        kernel-tail Drain.  Emit the waits as individual EVSEM wait
        instructions instead, then a waitless drain."""

        def _drain_and_barrier(self, tick_clock, wait_clock):
            dummy = mybir.InstDrain(
                name=f"I-{self.nc.next_id()}", engine=mybir.EngineType.SP
            )
            wait_clock.add_sem_waits(
                dummy, ScopedClock({None: tick_clock.global_clock})
            )
            waits = dummy.sync_info.on_wait if dummy.sync_info is not None else []
            assert self.sems is not None
            handles = {h.name: h for h in self.sems.allocated().values()}
            for w in waits:
                self.nc.sync.wait_ge(handles[w.ant_name], w.wait_value)
            self.nc.sync.drain()
            self.nc.all_engine_barrier()
            popped = self.nc._tile_sem_poison_stack.pop()
            assert popped is self._sem_poison
            self.nc.clear_and_free_semaphores(list(self.sems.allocated().values()))
            self.nc.all_engine_barrier()

    nc = bass.Bass("TRN2", target_bir_lowering=False, debug=False)

    def din(name, shape, dt=None):
        return nc.dram_tensor(name, list(shape), dt or f32,
                              kind="ExternalInput").ap()

    attn_d = din("attn", (LS, LL), f32r)
    embg_d = din("embg", (LS, D), f32r)
    pe_d = din("pe", (N_IB, P, D))
    wq_d = din("wq", (NL, P, N_KB, D), mmdt)
    wk_d = din("wk", (NL, P, N_KB, D), mmdt)
    wv_d = din("wv", (NL, P, N_KB, D), mmdt)
    wo_d = din("wo", (NL, P, N_KB, D), mmdt)
    w1_d = din("w1", (NL, N_FB, P, N_KB, P), mmdt)
    w2_d = din("w2", (NL, N_FB, P, D), mmdt)
    b1_d = din("b1", (NL, P, N_FB))
    if use_ln_gb:
        lnw_d = din("lnw", (NL, 4, P, D))   # ln1_g, ln1_b, ln2_g, ln2_b replicated
    if use_b2:
        b2r_d = din("b2r", (NL, P, D))
    projr_d = din("projr", (N_PB, P, N_KB, PBW), mmdt)
    if use_mask_bias:
        maskbT_d = din("maskbT", (N_IB, P, 1))
    out_d = nc.dram_tensor("out", [LL, PS], f32, kind="ExternalOutput").ap()

    with PatchedTC(nc) as tc, ExitStack() as stk:
        const = stk.enter_context(tc.tile_pool(name="const", bufs=1))
        sb = stk.enter_context(tc.tile_pool(name="sb", bufs=2))
        pp_big = stk.enter_context(tc.tile_pool(name="pp_big", bufs=2, space="PSUM"))
        pp_med = stk.enter_context(tc.tile_pool(name="pp_med", bufs=4, space="PSUM"))
        pp_o = stk.enter_context(tc.tile_pool(name="pp_o", bufs=2, space="PSUM"))

        ident = const.tile([P, P], f32, tag="ident", name="ident")
        make_identity(nc, ident)
        epsb = const.tile([P, 1], f32, tag="epsb", name="epsb")
        nc.vector.memset(epsb[:], 1e-5)

        # persistent zero-padded per-head stationaries: rows of the "other"
        # head stay zero forever, only the live half is rewritten per layer
        kTp = [const.tile([P, LL], mmdt, tag=f"kTp{h}", name=f"kTp{h}")
               for h in range(NH)]
        for h in range(NH):
            if USE_BF16:
                nc.vector.memset(kTp[h][:], 0.0)
            else:
                nc.vector.memset(kTp[h][:].bitcast(f32), 0.0)
        ones_sb = const.tile([P, 1], mmdt, tag="ones", name="ones")
        if USE_BF16:
            nc.vector.memset(ones_sb[:], 1.0)
        else:
            nc.vector.memset(ones_sb[:].bitcast(f32), 1.0)

        pe_sb = [const.tile([P, D], f32, tag=f"pe{ib}", name=f"pe{ib}")
                 for ib in range(N_IB)]
        for ib in range(N_IB):
            nc.sync.dma_start(pe_sb[ib][:], pe_d[ib])
        attn_sb = const.tile([LS, LL], f32r, tag="attn", name="attn")
        nc.sync.dma_start(attn_sb[:], attn_d[:])
        embg_sb = const.tile([LS, D], f32r, tag="embg", name="embg")
        nc.sync.dma_start(embg_sb[:], embg_d[:])
        if use_mask_bias:
            maskbT_sb = [const.tile([P, 1], f32, tag=f"maskbT{j}", name=f"maskbT{j}")
                         for j in range(N_IB)]
            for j in range(N_IB):
                nc.sync.dma_start(maskbT_sb[j][:], maskbT_d[j])

        def transpose_into(src_tiles, tag):
            """src: list of 2 [128, 512] seq-major f32 tiles -> 4 [128, 256]
            d-major f32r tiles."""
            dst = [sb.tile([P, LL], mmdt, tag=f"{tag}{kb}", name=f"{tag}{kb}",
                           bufs=2) for kb in range(N_KB)]
            for kb in range(N_KB):
                for ib in range(N_IB):
                    pt = pp_med.tile([P, 512], f32, tag="med", name="med")[:, :P]
                    nc.tensor.transpose(pt, src_tiles[ib][:, kb * P:(kb + 1) * P],
                                        ident[:])
                    nc.vector.tensor_copy(dst[kb][:, ib * P:(ib + 1) * P], pt)
            return dst

        def layer_norm(ps_in, resid, gw, bw, out_tag, bufs):
            """y = LN(psum + resid); var via E[x^2] - mu^2."""
            xr = sb.tile([P, D], f32, tag=f"{out_tag}_pre", name=f"{out_tag}_pre",
                         bufs=2)
            nc.vector.tensor_tensor(xr[:], ps_in[:], resid[:], OP.add)
            xsum = sb.tile([P, 1], f32, tag="xsum", name="xsum", bufs=4)
            nc.vector.tensor_reduce(xsum[:], xr[:], axis=AX.X, op=OP.add)
            sq = sb.tile([P, D], f32, tag="sq", name="sq", bufs=2)
            ssq = sb.tile([P, 1], f32, tag="ssq", name="ssq", bufs=4)
            nc.scalar.activation(sq[:], xr[:], ACT.Square, accum_out=ssq[:])
            negmean = sb.tile([P, 1], f32, tag="negmean", name="negmean", bufs=4)
            nc.vector.tensor_scalar_mul(negmean[:], xsum[:], -1.0 / D)
            msq = sb.tile([P, 1], f32, tag="msq", name="msq", bufs=4)
            nc.vector.tensor_tensor(msq[:], negmean[:], negmean[:], OP.mult)
            var = sb.tile([P, 1], f32, tag="var", name="var", bufs=4)
            nc.vector.tensor_scalar(var[:], ssq[:], 1.0 / D, msq[:],
                                    OP.mult, OP.subtract)
            std = sb.tile([P, 1], f32, tag="std", name="std", bufs=4)
            nc.scalar.activation(std[:], var[:], ACT.Sqrt, bias=epsb[:])
            rstd = sb.tile([P, 1], f32, tag="rstd", name="rstd", bufs=4)
            nc.vector.reciprocal(rstd[:], std[:])
            y = sb.tile([P, D], f32, tag=out_tag, name=out_tag, bufs=bufs)
            nc.vector.tensor_scalar(y[:], xr[:], negmean[:], rstd[:],
                                    OP.add, OP.mult)
            if use_ln_gb:
                nc.vector.tensor_tensor(y[:], y[:], gw[:], OP.mult)
                nc.vector.tensor_tensor(y[:], y[:], bw[:], OP.add)
            return y

        # ---- stage 0: x = attn.T @ embg + pe ----
        x = [sb.tile([P, D], f32, tag="x", name="x", bufs=4) for _ in range(N_IB)]
        for ib in range(N_IB):
            ps_x = pp_big.tile([P, 512], f32, tag="mm", name="mm")
            nc.tensor.matmul(ps_x[:, :D], attn_sb[:, ib * P:(ib + 1) * P],
                             embg_sb[:], start=True, stop=True)
            nc.vector.tensor_tensor(x[ib][:], ps_x[:, :D], pe_sb[ib][:], OP.add)

        # ---- transformer layers ----
        for l in range(NL):
            wq_t = sb.tile([P, N_KB, D], mmdt, tag="wq", name="wq", bufs=2)
            nc.sync.dma_start(wq_t[:], wq_d[l])
            wk_t = sb.tile([P, N_KB, D], mmdt, tag="wk", name="wk", bufs=2)
            nc.sync.dma_start(wk_t[:], wk_d[l])
            wv_t = sb.tile([P, N_KB, D], mmdt, tag="wv", name="wv", bufs=2)
            nc.sync.dma_start(wv_t[:], wv_d[l])
            wo_t = sb.tile([P, N_KB, D], mmdt, tag="wo", name="wo", bufs=2)
            nc.sync.dma_start(wo_t[:], wo_d[l])
            b1_t = sb.tile([P, N_FB], f32, tag="b1", name="b1", bufs=2)
            nc.sync.dma_start(b1_t[:], b1_d[l])
            if use_ln_gb:
                ln_t = [sb.tile([P, D], f32, tag=f"lnw{j}", name=f"lnw{j}", bufs=2)
                        for j in range(4)]
                for j in range(4):
                    nc.sync.dma_start(ln_t[j][:], lnw_d[l, j])
            else:
                ln_t = [None] * 4
            if use_b2:
                b2_t = sb.tile([P, D], f32, tag="b2r", name="b2r", bufs=2)
                nc.sync.dma_start(b2_t[:], b2r_d[l])

            xT = transpose_into(x, "xT")

            # qT full tiles [128 (d), 256 (seq)] pre-scaled by 1/8 (ACT);
            # kT only as halves inside the zero-padded kTp (DVE)
            qT = []
            for nb in range(N_KB):
                ps_q = pp_med.tile([P, LL], f32, tag="med", name="med")
                for kb in range(N_KB):
                    nc.tensor.matmul(ps_q[:], wq_t[:, kb, nb * P:(nb + 1) * P],
                                     xT[kb][:],
                                     start=(kb == 0), stop=(kb == N_KB - 1))
                t = sb.tile([P, LL], mmdt, tag=f"qT{nb}", name=f"qT{nb}", bufs=2)
                nc.scalar.activation(t[:], ps_q[:], ACT.Copy,
                                     scale=1.0 / float(np.sqrt(DH)))
                qT.append(t)
            for nb in range(N_KB):
                ps_q = pp_med.tile([P, LL], f32, tag="med", name="med")
                for kb in range(N_KB):
                    nc.tensor.matmul(ps_q[:], wk_t[:, kb, nb * P:(nb + 1) * P],
                                     xT[kb][:],
                                     start=(kb == 0), stop=(kb == N_KB - 1))
                for half in range(2):
                    h = 2 * nb + half
                    sl = slice(DH * half, DH * (half + 1))
                    nc.vector.tensor_copy(kTp[h][sl, :], ps_q[sl, :])

            # v in normal layout: [128 (seq j), 512 (d)] x2
            v = []
            for ib in range(N_IB):
                ps_v = pp_big.tile([P, 512], f32, tag="mm", name="mm")
                for kb in range(N_KB):
                    nc.tensor.matmul(ps_v[:, :D], xT[kb][:, ib * P:(ib + 1) * P],
                                     wv_t[:, kb, :],
                                     start=(kb == 0), stop=(kb == N_KB - 1))
                t = sb.tile([P, D], mmdt, tag=f"v{ib}", name=f"v{ib}", bufs=2)
                nc.vector.tensor_copy(t[:], ps_v[:, :D])
                v.append(t)

            # attention: unnormalized exp; denominator folded into out-copy
            ps_o = [pp_o.tile([P, 512], f32, tag="po", name="po")
                    for _ in range(N_IB)]
            rinv = sb.tile([P, 2 * NH], f32, tag="rinv", name="rinv", bufs=2)
            for h in range(NH):
                ht = h // 2
                # sT (j on partitions) -> unnormalized attention, transposed
                uT = []
                for jb in range(N_IB):
                    ps_t = pp_med.tile([P, LL], f32, tag="med", name="med")
                    nc.tensor.matmul(ps_t[:], kTp[h][:, jb * P:(jb + 1) * P],
                                     qT[ht][:], start=True, stop=True)
                    if use_mask_bias:
                        nc.vector.tensor_scalar_add(ps_t[:], ps_t[:],
                                                    maskbT_sb[jb][:])
                    ut = sb.tile([P, LL], mmdt, tag="uT", name="uT", bufs=3)
                    nc.scalar.activation(ut[:], ps_t[:], ACT.Exp)
                    uT.append(ut)
                # o_unnorm[i, d_h] += uT.T @ v_h ; row sums via uT.T @ ones
                for ib in range(N_IB):
                    ps_r = pp_med.tile([P, 512], f32, tag="med", name="med")
                    for jb in range(N_IB):
                        nc.tensor.matmul(ps_o[ib][:, DH * h:DH * (h + 1)],
                                         uT[jb][:, ib * P:(ib + 1) * P],
                                         v[jb][:, DH * h:DH * (h + 1)],
                                         start=(jb == 0), stop=(jb == N_IB - 1))
                        nc.tensor.matmul(ps_r[:, :1],
                                         uT[jb][:, ib * P:(ib + 1) * P],
                                         ones_sb[:],
                                         start=(jb == 0), stop=(jb == N_IB - 1))
                    nc.vector.reciprocal(rinv[:, 2 * h + ib:2 * h + ib + 1],
                                         ps_r[:, :1])
            # normalize during psum -> sbuf copy (scale = 1/rowsum per head)
            o_sb = [sb.tile([P, D], f32, tag=f"o{ib}", name=f"o{ib}", bufs=2)
                    for ib in range(N_IB)]
            for ib in range(N_IB):
                for h in range(NH):
                    nc.scalar.activation(o_sb[ib][:, DH * h:DH * (h + 1)],
                                         ps_o[ib][:, DH * h:DH * (h + 1)],
                                         ACT.Copy,
                                         scale=rinv[:, 2 * h + ib:2 * h + ib + 1])
            oT = transpose_into(o_sb, "oT")

            # attn_out + residual -> LN1 -> x1
            x1 = []
            for ib in range(N_IB):
                ps_ao = pp_big.tile([P, 512], f32, tag="mm", name="mm")
                for kb in range(N_KB):
                    nc.tensor.matmul(ps_ao[:, :D], oT[kb][:, ib * P:(ib + 1) * P],
                                     wo_t[:, kb, :],
                                     start=(kb == 0), stop=(kb == N_KB - 1))
                x1.append(layer_norm(ps_ao[:, :D], x[ib], ln_t[0], ln_t[1],
                                     "x1", 2))

            x1T = transpose_into(x1, "x1T")

            # FFN fused: per fb compute hT[fb], then accumulate into ffn psums
            ps_f = [pp_big.tile([P, 512], f32, tag="mm", name="mm")
                    for _ in range(N_IB)]
            for fb in range(N_FB):
                w1_t = sb.tile([P, N_KB, P], mmdt, tag="w1t", name="w1t", bufs=4)
                nc.sync.dma_start(w1_t[:], w1_d[l, fb])
                w2_t = sb.tile([P, D], mmdt, tag="w2t", name="w2t", bufs=4)
                nc.sync.dma_start(w2_t[:], w2_d[l, fb])
                ps_h = pp_med.tile([P, LL], f32, tag="med", name="med")
                for kb in range(N_KB):
                    nc.tensor.matmul(ps_h[:], w1_t[:, kb, :], x1T[kb][:],
                                     start=(kb == 0), stop=(kb == N_KB - 1))
                hT = sb.tile([P, LL], mmdt, tag="hT", name="hT", bufs=3)
                nc.scalar.activation(hT[:], ps_h[:], ACT.Relu,
                                     bias=b1_t[:, fb:fb + 1])
                for ib in range(N_IB):
                    nc.tensor.matmul(ps_f[ib][:, :D], hT[:, ib * P:(ib + 1) * P],
                                     w2_t[:],
                                     start=(fb == 0), stop=(fb == N_FB - 1))

            x_next = []
            for ib in range(N_IB):
                if use_b2:
                    nc.vector.tensor_tensor(ps_f[ib][:, :D], ps_f[ib][:, :D],
                                            b2_t[:], OP.add)
                x_next.append(layer_norm(ps_f[ib][:, :D], x1[ib], ln_t[2],
                                         ln_t[3], "x", 4))
            x = x_next

        # ---- final projection: out = x @ projT ----
        xTf = transpose_into(x, "xT")
        for pb in range(N_PB):
            pj_t = sb.tile([P, N_KB, PBW], mmdt, tag="pjt", name="pjt", bufs=3)
            nc.sync.dma_start(pj_t[:], projr_d[pb])
            for ib in range(N_IB):
                ps_p = pp_big.tile([P, 512], f32, tag="mm", name="mm")
                for kb in range(N_KB):
                    nc.tensor.matmul(ps_p[:, :PBW], xTf[kb][:, ib * P:(ib + 1) * P],
                                     pj_t[:, kb, :],
                                     start=(kb == 0), stop=(kb == N_KB - 1))
                o_t = sb.tile([P, PBW], f32, tag="outsb", name="outsb", bufs=4)
                if (pb + ib) % 2 == 0:
                    nc.scalar.copy(o_t[:], ps_p[:, :PBW])
                else:
                    nc.vector.tensor_copy(o_t[:], ps_p[:, :PBW])
                nc.sync.dma_start(out_d[ib * P:(ib + 1) * P,
                                          pb * PBW:(pb + 1) * PBW], o_t[:])

    _split_excess_waits(nc, mybir, maxw=1)
    return nc


def _split_excess_waits(nc, mybir, maxw=1):
    """This container's walrus rejects instructions with more than `maxw`
    sync waits.  Move excess immediate sem waits onto standalone EVSEM
    instructions inserted just before, on the same engine."""
    for fn in nc.m.functions:
        for blk in fn.blocks:
            out = []
            changed = False
            for ins in blk.instructions:
                si = ins.sync_info
                if si is not None and len(si.on_wait) > maxw:
                    waits = list(si.on_wait)
                    movable = [w for w in waits
                               if w.sync_type == "semaphore" and w.wait_reg is None]
                    fixed = [w for w in waits if w not in movable]
                    keep_budget = maxw - len(fixed)
                    assert keep_budget >= 0, f"unmovable waits exceed limit: {ins}"
                    keep = movable[len(movable) - keep_budget:] if keep_budget else []
                    move = movable[:len(movable) - keep_budget]
                    for i in range(0, len(move), maxw):
                        ev = mybir.InstEventSemaphore(
                            name=f"I-{nc.next_id()}", engine=ins.engine)
                        ev.sync_info = mybir.SyncInfo(
                            on_wait=move[i:i + maxw], on_update=[])
                        nc.register_instruction(ev, overwrite=True)
                        out.append(ev)
                    ins.sync_info = mybir.SyncInfo(
                        on_wait=fixed + keep, on_update=list(si.on_update))
                    changed = True
                out.append(ins)
            if changed:
                blk.instructions = out


def _host_prepare(inputs):
    """Returns (shared_map, per_core_list, flags)."""
    g = {k: np.asarray(v) for k, v in inputs.items()}
    attn, mask, tgt = g["attn"], g["mask"], g["tgt"]
    emb_w, proj_w = np.asarray(g["emb_w"], F32), np.asarray(g["proj_w"], F32)

    # positional encoding (matches reference)
    pos = np.arange(LL, dtype=F32)[:, None]
    div = np.exp(np.arange(0, D, 2, dtype=F32) * (-np.log(10000.0) / D))
    pe = np.zeros((LL, D), F32)
    pe[:, 0::2] = np.sin(pos * div)
    pe[:, 1::2] = np.cos(pos * div)
    pe_r = np.ascontiguousarray(pe.reshape(N_IB, P, D))

    def kmajor(w):  # (512, N) -> (128, 4, N)
        n = w.shape[1]
        return np.ascontiguousarray(
            np.asarray(w, F32).reshape(N_KB, P, n).transpose(1, 0, 2))

    import ml_dtypes
    mmnp = ml_dtypes.bfloat16 if USE_BF16 else F32

    wq = np.stack([kmajor(g["Wq"][l]) for l in range(NL)])
    wk = np.stack([kmajor(g["Wk"][l]) for l in range(NL)])
    wv = np.stack([kmajor(g["Wv"][l]) for l in range(NL)])
    wo = np.stack([kmajor(g["Wo"][l]) for l in range(NL)])
    # W1: (512, 2048) -> (16, 128, 4, 128)
    w1 = np.stack([
        np.ascontiguousarray(
            np.asarray(g["W1"][l], F32).reshape(N_KB, P, N_FB, P)
            .transpose(2, 1, 0, 3))
        for l in range(NL)])
    # W2: (2048, 512) -> (16, 128, 512)
    w2 = np.stack([
        np.ascontiguousarray(np.asarray(g["W2"][l], F32).reshape(N_FB, P, D))
        for l in range(NL)])
    b1 = np.stack([
        np.ascontiguousarray(np.asarray(g["b1"][l], F32).reshape(N_FB, P).T)
        for l in range(NL)])
    projr = np.ascontiguousarray(
        proj_w.T.reshape(N_KB, P, N_PB, PBW).transpose(2, 1, 0, 3))

    ln_vecs = [np.asarray(g[k], F32) for k in ("ln1_g", "ln1_b", "ln2_g", "ln2_b")]
    use_ln_gb = not (np.all(ln_vecs[0] == 1) and np.all(ln_vecs[1] == 0)
                     and np.all(ln_vecs[2] == 1) and np.all(ln_vecs[3] == 0))
    use_b2 = bool(np.any(np.asarray(g["b2"], F32) != 0))
    use_mask_bias = not bool(np.asarray(mask).all())

    ones = np.ones((P, 1), F32)
    shared = dict(pe=pe_r, b1=b1,
                  wq=wq.astype(mmnp), wk=wk.astype(mmnp), wv=wv.astype(mmnp),
                  wo=wo.astype(mmnp), w1=w1.astype(mmnp), w2=w2.astype(mmnp),
                  projr=projr.astype(mmnp))
    if use_ln_gb:
        shared["lnw"] = np.stack([
            np.stack([ones * v[l][None, :] for v in ln_vecs])
            for l in range(NL)])
    if use_b2:
        shared["b2r"] = np.stack([ones * np.asarray(g["b2"][l], F32)[None, :]
                                  for l in range(NL)])

    per_core = []
    sqrt_d = np.sqrt(np.float32(D))
    for b in range(BS):
        tg = np.asarray(tgt[b]).astype(np.int64)
        embg = emb_w[tg] * (tg != 0)[:, None].astype(F32) * sqrt_d
        m = dict(attn=np.ascontiguousarray(np.asarray(attn[b], F32)),
                 embg=np.ascontiguousarray(embg.astype(F32)))
        if use_mask_bias:
            mb = np.where(np.asarray(mask[b]), 0.0, -1e9).astype(F32)
            m["maskbT"] = np.ascontiguousarray(mb.reshape(N_IB, P, 1))
        per_core.append(m)
    return shared, per_core, (use_mask_bias, use_ln_gb, use_b2)


def kernel(**inputs):
    from concourse import bass_utils

    shared, per_core, flags = _host_prepare(inputs)
    nc = _build_bass(*flags)
    in_maps = [{**shared, **pc} for pc in per_core]
    res = bass_utils.run_bass_kernel_spmd(nc, in_maps, core_ids=list(range(BS)),
                                          trace=False)
    out = np.stack([res.results[b]["out"] for b in range(BS)])
    return out.astype(F32)
